# revision 3
# baseline (speedup 1.0000x reference)
"""Trainium2 Bass kernel for a fake-quantized MLP (qlinear -> gelu -> qlinear).

Reference semantics (B,S,C,H = 32,1024,1024,4096):
    x2d = x.reshape(-1, C)
    h   = round(x2d/sx) @ round(w1/sw1).T * (sx*sw1) + b1 ;  s = max(amax,eps)/127
    g   = gelu(h, exact erf)
    y   = round(g/sh) @ round(w2/sw2).T * (sh*sw2) + b2

v2 strategy (data-parallel over rows, 8 cores):
  * fp16 everywhere off the PE: quantized ints (|q|<=127) are exact in fp16.
    Rounding must happen in f32 ARITHMETIC (ACT pass: x*inv_s + 1.5*2**23,
    in place) because the engines' f32->fp16 output conversion truncates;
    the DVE -MAGIC pass then emits the exact integer as fp16.
  * No PE transposes except w1 (front): quantized rows are staged to DRAM in
    fp16 and read back transposed with single-instruction XBAR DMA
    transposes, so phase A / B PE work is pure matmul.
  * gelu output g is staged to DRAM as fp16 (validated: adds <1e-4 rel err),
    halving the h roundtrip traffic; phase B reads it back with one DMA per
    row chunk and requantizes in place.
  * Partition-dim max-reductions go through a tiny DRAM roundtrip on the
    gpsimd queue instead of PE-identity matmuls, so the AllReduce(max)
    triggers don't queue behind PE work.
  * The PE HAM clock gate drops to half rate after ~3.4us of idle; dummy
    accumulate matmuls ("junk warmers") keep it at 2.4GHz through the
    DMA-bound front and the mid-kernel AllReduce bubble.
  * Schedule: x amax scan first (its AllReduce hides under the w1 load),
    w1 SBUF-resident f32 so scan+quant reads it once; w2 scan trickles
    through phase A chunk 0, w2 quant through chunk 1, w2qT transposes land
    in the AllReduce(h) bubble; phase B starts with small row chunks to
    shorten the post-AllReduce critical path.
"""

import sys

import numpy as np

try:
    import concourse.bass as bass
except ImportError:  # pragma: no cover
    sys.path.insert(0, "/opt/trn_rl_repo")
    import concourse.bass as bass

import concourse.mybir as mybir
from contextlib import ExitStack
import concourse.tile as tile
from concourse import masks
from concourse.bass_utils import run_bass_kernel_spmd

from concourse.bass import _add_dep_helper as _add_dep

F32 = mybir.dt.float32
FP16 = mybir.dt.float16
AF = mybir.ActivationFunctionType
ALU = mybir.AluOpType

QP = 127.0
EPS = 1e-6
MAGIC = 12582912.0  # 1.5 * 2**23: f32 round-to-nearest-even integer trick

# full problem shapes
B, S, C, H = 32, 1024, 1024, 4096
N_CORES = 8

CH = 2048            # phase A row chunk
W1_RES = 30          # w1 row-blocks kept f32-resident in SBUF (of 32)
JUNK_FRONT = 480     # PE warmers covering the DMA/AllReduce-bound front
JUNK_MID = 170       # PE warmers covering the AllReduce(h) bubble
# phase B row chunks: small first to shorten the post-AllReduce critical path
PHB_CHUNKS = [128, 128, 256] + [512] * 7


def _split_matmul_waits(nc):
    """This toolchain's walrus codegen allows only ONE sync-wait slot per
    lowered instruction (Matmult waits all land on its LDWEIGHTS since
    --enable-ldw-opt=false; queue DMAs use a single-slot DIRECT2D struct).
    Peel extra waits onto same-engine NoOps inserted just before, except for
    framework-generated drain/barrier instructions which support many."""
    n_split = 0
    for f in nc.m.functions:
        for bb in f.blocks:
            insts = bb.instructions
            out = []
            changed = False
            for inst in insts:
                si = getattr(inst, "sync_info", None)
                if si is not None and si.on_wait and len(si.on_wait) > 1:
                    waits = list(si.on_wait)
                    for k, w in enumerate(waits[:-1]):
                        nop = mybir.InstNoOp(
                            name=f"{inst.name}-wsplit{k}", ins=[], outs=[]
                        )
                        nop.engine = inst.engine
                        nop.sync_info = mybir.SyncInfo(
                            on_wait=[w], on_update=[]
                        )
                        out.append(nop)
                    inst.sync_info = mybir.SyncInfo(
                        on_wait=[waits[-1]], on_update=list(si.on_update or [])
                    )
                    n_split += 1
                    changed = True
                out.append(inst)
            if changed:
                bb.instructions = out
    return n_split


def _dedup_ldweights(nc):
    """Tile legalization emits explicit Ldweights+Matmult pairs, and walrus
    runs with --enable-ldw-opt=false, so every matmul re-streams its
    stationary operand (128 extra PE cycles per matmul).  Drop an Ldweights
    whose weights AP is identical to the previous one on the PE stream (the
    PE array still holds that stationary); keep its semaphore effects on a
    NoOp."""
    n = 0
    for f in nc.m.functions:
        for bb in f.blocks:
            insts = bb.instructions
            out = []
            last_key = None
            changed = False
            for inst in insts:
                if isinstance(inst, mybir.InstLdweights):
                    key = str(inst.ins[0])
                    if key == last_key:
                        si = getattr(inst, "sync_info", None)
                        if si is not None and (si.on_wait or si.on_update):
                            nop = mybir.InstNoOp(
                                name=inst.name + "-lw", ins=[], outs=[]
                            )
                            nop.engine = inst.engine
                            nop.sync_info = si
                            out.append(nop)
                        n += 1
                        changed = True
                        continue
                    last_key = key
                elif isinstance(inst, mybir.InstMatmult):
                    if inst.is_transpose or getattr(inst, "ldweights", None):
                        last_key = None
                out.append(inst)
            if changed:
                bb.instructions = out
    return n


def build_nc(rows=4096, c=C, h=H, n_cores=N_CORES, gelu="Gelu",
             split_waits=True):
    """Build the per-core SPMD Bass program.

    rows: rows of x2d handled by each core.
    gelu: "Gelu" (HW ACT table), "Erf" (x*(0.5*erf(x/sqrt2)+0.5)),
          "Identity" (for simulator runs; CoreSim lacks Gelu/Erf).
    """
    assert rows % CH == 0 and c % 512 == 0 and h % 512 == 0
    nc = bass.Bass()

    x_in = nc.dram_tensor("x", [rows, c], F32, kind="ExternalInput")
    w1_in = nc.dram_tensor("w1", [h, c], F32, kind="ExternalInput")
    b1_in = nc.dram_tensor("b1", [h], F32, kind="ExternalInput")
    w2_in = nc.dram_tensor("w2", [c, h], F32, kind="ExternalInput")
    b2_in = nc.dram_tensor("b2", [c], F32, kind="ExternalInput")
    y_out = nc.dram_tensor("y", [rows, c], F32, kind="ExternalOutput")

    ct = c // 128    # 8
    ht = h // 128    # 32
    n_chA = rows // CH
    groups = [list(range(n_cores))]
    phb_chunks = []
    m0 = 0
    for ln in PHB_CHUNKS:
        phb_chunks.append((m0, ln))
        m0 += ln
    assert m0 == rows

    with tile.TileContext(nc) as tc, ExitStack() as top:
        consts = top.enter_context(tc.tile_pool(name="consts", bufs=1))
        scal = top.enter_context(tc.tile_pool(name="scal", bufs=1))
        dram = top.enter_context(tc.tile_pool(name="dram", bufs=1, space="DRAM"))

        ident = consts.tile([128, 128], FP16)
        masks.make_identity(nc, ident[:])
        # b1 as (128, ht): b1_sb[p, jb] = b1[jb*128 + p]
        b1_sb = consts.tile([128, ht], F32)
        nc.sync.dma_start(
            out=b1_sb[:], in_=b1_in.ap().rearrange("(a b) -> b a", b=128)
        )
        magic_b = consts.tile([128, 1], F32)
        nc.vector.memset(magic_b[:], MAGIC)
        junk_sb = consts.tile([128, 512], FP16)
        nc.vector.memset(junk_sb[:], 1.0)

        # DRAM scratch
        xq_dram = dram.tile([rows, c], FP16)
        g_dram = dram.tile([h, rows], FP16)
        w2q_dram = dram.tile([c, h], FP16)
        rt_dram = dram.tile([4, 128], F32, tag="rt")    # partition-red rows
        sc_dram = dram.tile([2, 1], F32, tag="sc")      # w1max, w2max scalars
        arx_in = dram.tile([1, 1], F32, tag="arxi")
        arx_out = dram.tile([1, 1], F32, tag="arxo")
        arh_in = dram.tile([1, 1], F32, tag="arhi")
        arh_out = dram.tile([1, 1], F32, tag="arho")

        rowp = top.enter_context(tc.tile_pool(name="rowp", bufs=2))

        def _preduce_dma(acc, slot, tag):
            """(128,1) partition max -> (1,1), via DRAM roundtrip on the
            gpsimd software queue (keeps PE + the HWDGE queues out of the
            scale critical path)."""
            nc.gpsimd.dma_start(out=rt_dram[slot:slot + 1, :], in_=acc[:])
            row = rowp.tile([1, 128], F32, tag="rr", name=f"row_{tag}")
            nc.gpsimd.dma_start(out=row[:], in_=rt_dram[slot:slot + 1, :])
            s = scal.tile([1, 1], F32, name=f"s1_{tag}")
            nc.vector.tensor_reduce(
                out=s[:], in_=row[:], axis=mybir.AxisListType.X, op=ALU.max
            )
            return s

        def _derive(bcast_src_dram, name):
            b = scal.tile([128, 1], F32, name=name + "_b")
            nc.gpsimd.dma_start(
                out=b[:], in_=bcast_src_dram.to_broadcast((128, 1))
            )
            s = scal.tile([128, 1], F32, name="s_" + name)
            nc.vector.tensor_scalar(
                out=s[:], in0=b[:], scalar1=EPS, scalar2=float(1.0 / QP),
                op0=ALU.max, op1=ALU.mult,
            )
            inv = scal.tile([128, 1], F32, name="inv_" + name)
            nc.vector.reciprocal(out=inv[:], in_=s[:])
            return s, inv

        def _junk(n, pspool, tag):
            """PE warmers: n chained 512-wide accumulating matmuls on a
            constant tile.  Ldweights dedup leaves a single stationary load,
            so each costs ~515 PE cycles and keeps the HAM clock gate at
            K=8 while real work is DMA/collective-bound."""
            ps = pspool.tile([128, 512], F32, tag=tag, name=f"ps_{tag}")
            prev = None
            for i in range(n):
                mmi = nc.tensor.matmul(
                    ps[:], lhsT=junk_sb[:, 0:128], rhs=junk_sb[:],
                    start=(i == 0), stop=(i == n - 1),
                    skip_group_check=True,
                )
                if prev is not None:
                    _add_dep(mmi.ins, prev.ins, sync=False, reason="junk-order")
                prev = mmi
            dr = rowp.tile([128, 1], F32, tag="jdr", name=f"jdr_{tag}")
            nc.vector.tensor_reduce(
                out=dr[:], in_=ps[:], axis=mybir.AxisListType.X, op=ALU.max
            )
            return prev

        # w1qT outlives the front, dies after phase A
        w1_stack = ExitStack()
        w1qT_pool = w1_stack.enter_context(
            tc.tile_pool(name="w1qT", bufs=1, side="right")
        )
        w1qT = [
            w1qT_pool.tile([128, h], FP16, tag=f"w1qT{i}", name=f"w1qT{i}")
            for i in range(ct)
        ]

        # ---------------- front ----------------
        xmax = scal.tile([128, 1], F32)
        nc.vector.memset(xmax[:], 0.0)
        wmax1 = scal.tile([128, 1], F32)
        nc.vector.memset(wmax1[:], 0.0)

        with ExitStack() as front:
            psJ = front.enter_context(
                tc.tile_pool(name="psJ", bufs=1, space="PSUM")
            )
            xs = front.enter_context(tc.tile_pool(name="xs", bufs=2))
            xr = front.enter_context(tc.tile_pool(name="xr", bufs=2))
            w1f_pool = front.enter_context(tc.tile_pool(name="w1f", bufs=1))
            wq1 = front.enter_context(tc.tile_pool(name="wq1", bufs=2))
            psT = front.enter_context(
                tc.tile_pool(name="psT", bufs=4, space="PSUM")
            )

            _junk(JUNK_FRONT, psJ, "junkF")

            # x amax scan (sharded -> AllReduce max) FIRST: the AllReduce
            # latency hides under the w1 load that follows on the same queue.
            for t in range(rows // 128):
                xt = xs.tile([128, c], F32, tag="xs", name=f"xs{t}")
                nc.sync.dma_start(out=xt[:], in_=x_in[t * 128:(t + 1) * 128, :])
                r = xr.tile([128, 1], F32, tag="xr", name=f"xr{t}")
                nc.vector.tensor_reduce(
                    out=r[:], in_=xt[:], axis=mybir.AxisListType.X, op=ALU.max,
                    apply_absolute_value=True,
                )
                nc.vector.tensor_tensor(
                    out=xmax[:], in0=xmax[:], in1=r[:], op=ALU.max
                )
            xm_s = _preduce_dma(xmax, 0, "xm")
            nc.gpsimd.dma_start(out=arx_in[:], in_=xm_s[:])
            nc.gpsimd.collective_compute(
                "AllReduce", ALU.max, replica_groups=groups,
                ins=[arx_in.opt()], outs=[arx_out.opt()],
            )

            # w1 load (resident f32 where possible) + amax; every core scans
            # the FULL weights so the local max is already global.
            w1res = []
            for t in range(ht):
                if t < W1_RES:
                    wt = w1f_pool.tile([128, c], F32, tag=f"w1f{t}",
                                       name=f"w1f{t}")
                    w1res.append(wt)
                else:
                    wt = xs.tile([128, c], F32, tag="xs", name=f"w1s{t}")
                nc.sync.dma_start(out=wt[:], in_=w1_in[t * 128:(t + 1) * 128, :])
                r = xr.tile([128, 1], F32, tag="xr", name=f"w1r{t}")
                nc.vector.tensor_reduce(
                    out=r[:], in_=wt[:], axis=mybir.AxisListType.X, op=ALU.max,
                    apply_absolute_value=True,
                )
                nc.vector.tensor_tensor(
                    out=wmax1[:], in0=wmax1[:], in1=r[:], op=ALU.max
                )
            w1m_s = _preduce_dma(wmax1, 1, "w1m")
            nc.gpsimd.dma_start(out=sc_dram[0:1, :], in_=w1m_s[:])
            sw1, inv_sw1 = _derive(sc_dram[0:1, :], "w1")
            sx, inv_sx = _derive(arx_out, "x")
            sxw1 = scal.tile([128, 1], F32)
            nc.vector.tensor_tensor(out=sxw1[:], in0=sx[:], in1=sw1[:],
                                    op=ALU.mult)

            # w1 quant + PE transpose: ACT rounds in f32 (in place), DVE
            # casts the exact ints to fp16, PE transposes the int rows via
            # identity matmul (exact in f32 PSUM), DVE copy-drains.
            for t in range(ht):
                if t < W1_RES:
                    src = w1res[t]
                else:
                    src = xs.tile([128, c], F32, tag="xs", name=f"w1q_s{t}")
                    nc.sync.dma_start(
                        out=src[:], in_=w1_in[t * 128:(t + 1) * 128, :]
                    )
                nc.scalar.activation(
                    out=src[:], in_=src[:], func=AF.Identity, bias=magic_b[:],
                    scale=inv_sw1[:],
                )
                q1 = wq1.tile([128, c], FP16, tag="wq1", name=f"w1q1_{t}")
                nc.vector.tensor_scalar_add(out=q1[:], in0=src[:],
                                            scalar1=-MAGIC)
                for cb in range(ct):
                    ps = psT.tile([128, 128], F32, tag="psT",
                                  name=f"psT{t}_{cb}")
                    nc.tensor.matmul(
                        ps[:], lhsT=q1[:, cb * 128:(cb + 1) * 128],
                        rhs=ident[:], start=True, stop=True,
                    )
                    nc.vector.tensor_copy(
                        out=w1qT[cb][:, t * 128:(t + 1) * 128], in_=ps[:]
                    )

        # ---------------- phase A ----------------
        hmax = scal.tile([128, 1], F32)
        nc.vector.memset(hmax[:], 0.0)
        wmax2 = scal.tile([128, 1], F32)
        nc.vector.memset(wmax2[:], 0.0)
        sw2 = inv_sw2 = None

        with ExitStack() as pha:
            xqT_pool = pha.enter_context(tc.tile_pool(name="xqT", bufs=2))
            psH = pha.enter_context(
                tc.tile_pool(name="psH", bufs=8, space="PSUM")
            )
            xf = pha.enter_context(tc.tile_pool(name="xf", bufs=3))
            xq1 = pha.enter_context(tc.tile_pool(name="xq1", bufs=3))
            gS = pha.enter_context(tc.tile_pool(name="gS", bufs=3))
            gr = pha.enter_context(tc.tile_pool(name="gr", bufs=3))
            w2f = pha.enter_context(tc.tile_pool(name="w2f", bufs=2))
            w2qs = pha.enter_context(tc.tile_pool(name="w2qs", bufs=2))

            for mc in range(n_chA):
                # x quant: load f32 rows, magic-round to fp16 ints, stage to
                # DRAM for the XBAR transpose read-back
                for t in range(CH // 128):
                    r0 = mc * CH + t * 128
                    xt = xf.tile([128, c], F32, tag="xf", name=f"xf{mc}_{t}")
                    nc.sync.dma_start(out=xt[:], in_=x_in[r0:r0 + 128, :])
                    nc.scalar.activation(
                        out=xt[:], in_=xt[:], func=AF.Identity,
                        bias=magic_b[:], scale=inv_sx[:],
                    )
                    q = xq1.tile([128, c], FP16, tag="xq1", name=f"xq{mc}_{t}")
                    nc.vector.tensor_scalar_add(out=q[:], in0=xt[:],
                                                scalar1=-MAGIC)
                    nc.sync.dma_start(out=xq_dram[r0:r0 + 128, :], in_=q[:])
                # XBAR transpose read-back, per (512-row quarter, cb) so the
                # first matmuls unblock as soon as quarter 0 lands
                xqTs = [
                    xqT_pool.tile([128, CH], FP16, tag=f"xqT{cb}",
                                  name=f"xqT{mc}_{cb}")
                    for cb in range(ct)
                ]
                for qt in range(CH // 512):
                    r0 = mc * CH + qt * 512
                    for cb in range(ct):
                        nc.sync.dma_start_transpose(
                            xqTs[cb][:, qt * 512:(qt + 1) * 512],
                            xq_dram[r0:r0 + 512, cb * 128:(cb + 1) * 128],
                        )

                # w2 trickle on the scalar HWDGE queue: scan during chunk 0,
                # quant+stage during chunk 1
                if mc == 0:
                    for t in range(16):
                        blk, hf = t // 2, t % 2
                        wt = w2f.tile([128, 2048], F32, tag="w2f",
                                      name=f"w2s{t}")
                        nc.scalar.dma_start(
                            out=wt[:],
                            in_=w2_in[blk * 128:(blk + 1) * 128,
                                      hf * 2048:(hf + 1) * 2048],
                        )
                        r = gr.tile([128, 1], F32, tag="gr", name=f"w2r{t}")
                        nc.vector.tensor_reduce(
                            out=r[:], in_=wt[:], axis=mybir.AxisListType.X,
                            op=ALU.max, apply_absolute_value=True,
                        )
                        nc.vector.tensor_tensor(
                            out=wmax2[:], in0=wmax2[:], in1=r[:], op=ALU.max
                        )
                else:
                    w2m_s = _preduce_dma(wmax2, 2, "w2m")
                    nc.gpsimd.dma_start(out=sc_dram[1:2, :], in_=w2m_s[:])
                    sw2, inv_sw2 = _derive(sc_dram[1:2, :], "w2")
                    for t in range(16):
                        blk, hf = t // 2, t % 2
                        wt = w2f.tile([128, 2048], F32, tag="w2f",
                                      name=f"w2q_s{t}")
                        nc.scalar.dma_start(
                            out=wt[:],
                            in_=w2_in[blk * 128:(blk + 1) * 128,
                                      hf * 2048:(hf + 1) * 2048],
                        )
                        nc.scalar.activation(
                            out=wt[:], in_=wt[:], func=AF.Identity,
                            bias=magic_b[:], scale=inv_sw2[:],
                        )
                        q = w2qs.tile([128, 2048], FP16, tag="w2qs",
                                      name=f"w2q{t}")
                        nc.vector.tensor_scalar_add(out=q[:], in0=wt[:],
                                                    scalar1=-MAGIC)
                        nc.scalar.dma_start(
                            out=w2q_dram[blk * 128:(blk + 1) * 128,
                                         hf * 2048:(hf + 1) * 2048],
                            in_=q[:],
                        )

                # matmuls: h.T chunk = w1q @ xq.T, gelu fused on drain
                for jb in range(ht):
                    phs = [
                        psH.tile([128, 512], F32, tag="psH",
                                 name=f"psH{mc}_{jb}_{i}")
                        for i in range(CH // 512)
                    ]
                    prev = None
                    for cb in range(ct):
                        for ms in range(CH // 512):
                            mmi = nc.tensor.matmul(
                                phs[ms][:],
                                lhsT=w1qT[cb][:, jb * 128:(jb + 1) * 128],
                                rhs=xqTs[cb][:, ms * 512:(ms + 1) * 512],
                                start=(cb == 0),
                                stop=(cb == ct - 1),
                            )
                            if prev is not None:
                                _add_dep(mmi.ins, prev.ins, sync=False,
                                         reason="ldw-order")
                            prev = mmi
                    g = gS.tile([128, CH], FP16, tag="gS", name=f"g{mc}_{jb}")
                    for ms in range(CH // 512):
                        if gelu == "Erf":
                            hh = gS.tile([128, 512], F32, tag="gHH",
                                         name=f"hh{mc}_{jb}_{ms}")
                            nc.scalar.activation(
                                out=hh[:], in_=phs[ms][:], func=AF.Identity,
                                bias=b1_sb[:, jb:jb + 1], scale=sxw1[:],
                            )
                            e = gS.tile([128, 512], F32, tag="gE",
                                        name=f"e{mc}_{jb}_{ms}")
                            nc.scalar.activation(
                                out=e[:], in_=hh[:], func=AF.Erf, bias=0.0,
                                scale=float(1.0 / np.sqrt(2.0)),
                            )
                            nc.vector.tensor_scalar(
                                out=e[:], in0=e[:], scalar1=0.5, scalar2=0.5,
                                op0=ALU.mult, op1=ALU.add,
                            )
                            nc.vector.tensor_tensor(
                                out=g[:, ms * 512:(ms + 1) * 512], in0=e[:],
                                in1=hh[:], op=ALU.mult,
                            )
                        else:
                            nc.scalar.activation(
                                out=g[:, ms * 512:(ms + 1) * 512],
                                in_=phs[ms][:], func=getattr(AF, gelu),
                                bias=b1_sb[:, jb:jb + 1], scale=sxw1[:],
                            )
                    r = gr.tile([128, 1], F32, tag="gr", name=f"gr{mc}_{jb}")
                    nc.vector.tensor_reduce(
                        out=r[:], in_=g[:], axis=mybir.AxisListType.X,
                        op=ALU.max, apply_absolute_value=True,
                    )
                    nc.vector.tensor_tensor(
                        out=hmax[:], in0=hmax[:], in1=r[:], op=ALU.max
                    )
                    nc.sync.dma_start(
                        out=g_dram[jb * 128:(jb + 1) * 128,
                                   mc * CH:(mc + 1) * CH],
                        in_=g[:],
                    )

        w1_stack.close()

        # ---------------- h scale AllReduce + transition ----------------
        hm_s = _preduce_dma(hmax, 3, "hm")
        nc.gpsimd.dma_start(out=arh_in[:], in_=hm_s[:])
        nc.gpsimd.collective_compute(
            "AllReduce", ALU.max, replica_groups=groups,
            ins=[arh_in.opt()], outs=[arh_out.opt()],
        )

        # ---------------- phase B ----------------
        with ExitStack() as phb:
            psY = phb.enter_context(
                tc.tile_pool(name="psY", bufs=6, space="PSUM")
            )
            psJ2 = phb.enter_context(
                tc.tile_pool(name="psJ2", bufs=1, space="PSUM")
            )
            w2qT_pool = phb.enter_context(tc.tile_pool(name="w2qT", bufs=1))
            hld = phb.enter_context(tc.tile_pool(name="hld", bufs=2))
            hq1p = phb.enter_context(tc.tile_pool(name="hq1p", bufs=3))
            yS = phb.enter_context(tc.tile_pool(name="yS", bufs=3))
            b2p = phb.enter_context(tc.tile_pool(name="b2p", bufs=1))

            # PE warmers across the AllReduce bubble (emitted after the last
            # phase-A matmul in PE program order)
            _junk(JUNK_MID, psJ2, "junkM")

            b2_b = b2p.tile([128, c], F32)
            nc.sync.dma_start(
                out=b2_b[:],
                in_=b2_in.ap().rearrange("(o a) -> o a", o=1).to_broadcast(
                    (128, c)),
            )
            # w2qT via XBAR transpose from the fp16 staging written in
            # phase A chunk 1 -- fills the AllReduce bubble on the DMA side
            w2qT = [
                w2qT_pool.tile([128, c], FP16, tag=f"w2qT{jb}",
                               name=f"w2qT{jb}")
                for jb in range(ht)
            ]
            for jb in range(ht):
                nc.sync.dma_start_transpose(
                    w2qT[jb][:], w2q_dram[:, jb * 128:(jb + 1) * 128]
                )

            sh, inv_sh = _derive(arh_out, "h")
            shw2 = scal.tile([128, 1], F32)
            nc.vector.tensor_tensor(out=shw2[:], in0=sh[:], in1=sw2[:],
                                    op=ALU.mult)

            for ci, (m0, mlen) in enumerate(phb_chunks):
                hl = hld.tile([128, ht, 512], FP16, tag="hld",
                              name=f"hl{ci}")
                hlv = hl[:, :, 0:mlen]
                nc.sync.dma_start(
                    out=hlv,
                    in_=g_dram[:, m0:m0 + mlen].rearrange(
                        "(a p) m -> p a m", p=128),
                )
                for j4 in range(ht // 4):
                    sl = hl[:, j4 * 4:(j4 + 1) * 4, 0:mlen]
                    hq1 = hq1p.tile([128, 4, 512], F32, tag="hq1",
                                    name=f"hq1_{ci}_{j4}")
                    nc.scalar.activation(
                        out=hq1[:, :, 0:mlen], in_=sl, func=AF.Identity,
                        bias=magic_b[:], scale=inv_sh[:],
                    )
                    nc.vector.tensor_scalar_add(
                        out=sl, in0=hq1[:, :, 0:mlen], scalar1=-MAGIC
                    )
                for ms in range(mlen // 128):
                    psa = psY.tile([128, 512], F32, tag="psY",
                                   name=f"psa{ci}_{ms}")
                    psb = psY.tile([128, 512], F32, tag="psY",
                                   name=f"psb{ci}_{ms}")
                    prev = None
                    for jb in range(ht):
                        lt = hl[:, jb:jb + 1, ms * 128:(ms + 1) * 128]
                        for ob, pso in ((0, psa), (1, psb)):
                            mmi = nc.tensor.matmul(
                                pso[:], lhsT=lt,
                                rhs=w2qT[jb][:, ob * 512:(ob + 1) * 512],
                                start=(jb == 0), stop=(jb == ht - 1),
                            )
                            if prev is not None:
                                _add_dep(mmi.ins, prev.ins, sync=False,
                                         reason="ldw-order")
                            prev = mmi
                    yt = yS.tile([128, c], F32, tag="yS", name=f"y{ci}_{ms}")
                    nc.vector.scalar_tensor_tensor(
                        out=yt[:, 0:512], in0=psa[:], scalar=shw2[:],
                        in1=b2_b[:, 0:512], op0=ALU.mult, op1=ALU.add,
                    )
                    nc.vector.scalar_tensor_tensor(
                        out=yt[:, 512:1024], in0=psb[:], scalar=shw2[:],
                        in1=b2_b[:, 512:1024], op0=ALU.mult, op1=ALU.add,
                    )
                    r0 = m0 + ms * 128
                    nc.sync.dma_start(out=y_out[r0:r0 + 128, :], in_=yt[:])

    if split_waits:
        _split_matmul_waits(nc)
        _dedup_ldweights(nc)
    return nc


_CACHED = {}


def _get_nc(rows, c, h, n_cores, gelu):
    key = (rows, c, h, n_cores, gelu)
    if key not in _CACHED:
        _CACHED[key] = build_nc(rows=rows, c=c, h=h, n_cores=n_cores,
                                gelu=gelu)
    return _CACHED[key]


def run(inputs, trace=False, gelu="Gelu", n_cores=N_CORES):
    x = np.asarray(inputs["x"], np.float32)
    w1 = np.ascontiguousarray(np.asarray(inputs["w1"], np.float32))
    b1 = np.ascontiguousarray(np.asarray(inputs["b1"], np.float32))
    w2 = np.ascontiguousarray(np.asarray(inputs["w2"], np.float32))
    b2 = np.ascontiguousarray(np.asarray(inputs["b2"], np.float32))
    b_, s_, c_ = x.shape
    h_ = w1.shape[0]
    x2d = np.ascontiguousarray(x.reshape(-1, c_))
    rows = x2d.shape[0] // n_cores
    nc = _get_nc(rows, c_, h_, n_cores, gelu)
    in_maps = [
        {
            "x": np.ascontiguousarray(x2d[i * rows:(i + 1) * rows]),
            "w1": w1,
            "b1": b1,
            "w2": w2,
            "b2": b2,
        }
        for i in range(n_cores)
    ]
    res = run_bass_kernel_spmd(nc, in_maps, list(range(n_cores)), trace=trace)
    y2d = np.concatenate([r["y"] for r in res.results], axis=0)
    return y2d.reshape(b_, s_, c_).astype(np.float32), res


def kernel(x, w1, b1, w2, b2):
    y, _ = run({"x": x, "w1": w1, "b1": b1, "w2": w2, "b2": b2})
    return y


# revision 7
# speedup vs baseline: 1.0852x; 1.0852x over previous
"""Trainium2 Bass kernel for a fake-quantized MLP (qlinear -> gelu -> qlinear).

Reference semantics (B,S,C,H = 32,1024,1024,4096):
    x2d = x.reshape(-1, C)
    h   = round(x2d/sx) @ round(w1/sw1).T * (sx*sw1) + b1 ;  s = max(amax,eps)/127
    g   = gelu(h, exact erf)
    y   = round(g/sh) @ round(w2/sw2).T * (sh*sw2) + b2

v3 strategy (data-parallel over rows, 8 cores):
  * fp16 everywhere off the PE: quantized ints (|q|<=127) are exact in fp16.
    Rounding happens in f32 ARITHMETIC (ACT pass: x*inv_s + 1.5*2**23, in
    place) because the engines' f32->fp16 output conversion truncates; the
    DVE -MAGIC pass then emits the exact integer as fp16.
  * Only w1 is transposed on the PE (front, overlapping the scans).  x and
    w2 quantized rows are staged to DRAM fp16 and read back transposed with
    single-instruction XBAR DMA transposes, so phase A/B PE work is pure
    matmul.  gelu output g is staged fp16 (adds <1e-4 rel err), halving the
    h roundtrip traffic.
  * Engines are strictly in-order: nothing slow may be emitted ahead of a
    time-critical op on the same engine.  Partition reductions roundtrip
    through DRAM on the gpsimd software DMA queue (walrus lowers only DMA
    for gpsimd); the junk-warmer psums are intentionally never read so no
    engine waits on them.
  * Both HWDGE queues are used: sync carries x/w1 scans (even tiles), g
    stores, phase-B h loads; scalar carries odd scan tiles, w2 traffic and
    the w2qT transposes so they cannot delay the phase-B h load.
  * The PE HAM clock gate drops to half rate after ~3.4us idle; chained
    dummy matmuls cover the DMA/AllReduce-bound front and the mid-kernel
    AllReduce bubble.  AllReduces are 4-byte max ops triggered from gpsimd
    (measured ~8us).
  * Phase A runs row chunks of 1024/1024/2048: small first chunks need only
    the first XBAR quarters, so matmuls start while x quant still streams;
    the last chunk amortizes ldweights over 4x512 moving.  Phase B chunks
    are 128/128/256/512... so the post-AllReduce critical path is short.
"""

import sys

import numpy as np

try:
    import concourse.bass as bass
except ImportError:  # pragma: no cover
    sys.path.insert(0, "/opt/trn_rl_repo")
    import concourse.bass as bass

import concourse.mybir as mybir
from contextlib import ExitStack
import concourse.tile as tile
from concourse import masks
from concourse import bass_isa
from concourse.bass_utils import run_bass_kernel_spmd

from concourse.bass import _add_dep_helper as _add_dep

F32 = mybir.dt.float32
FP16 = mybir.dt.float16
AF = mybir.ActivationFunctionType
ALU = mybir.AluOpType

QP = 127.0
EPS = 1e-6
MAGIC = 12582912.0  # 1.5 * 2**23: f32 round-to-nearest-even integer trick

# full problem shapes
B, S, C, H = 32, 1024, 1024, 4096
N_CORES = 8

A_CHUNKS = [1024, 1024, 2048]   # phase A row chunks
W1_RES = 14          # w1 row-blocks kept f32-resident in SBUF (of 32)
JUNK_FRONT = 260     # PE warmers covering the DMA/AllReduce-bound front
JUNK_MID = 120       # PE warmers covering the AllReduce(h) bubble
# phase B row chunks: small first to shorten the post-AllReduce critical path
PHB_CHUNKS = [128, 128, 256] + [512] * 7


def _split_matmul_waits(nc):
    """This toolchain's walrus codegen allows only ONE sync-wait slot per
    lowered instruction (Matmult waits all land on its LDWEIGHTS since
    --enable-ldw-opt=false; queue DMAs use a single-slot DIRECT2D struct).
    Peel extra waits onto same-engine NoOps inserted just before, except for
    framework-generated drain/barrier instructions which support many."""
    n_split = 0
    for f in nc.m.functions:
        for bb in f.blocks:
            insts = bb.instructions
            out = []
            changed = False
            for inst in insts:
                si = getattr(inst, "sync_info", None)
                if si is not None and si.on_wait and len(si.on_wait) > 1:
                    waits = list(si.on_wait)
                    for k, w in enumerate(waits[:-1]):
                        nop = mybir.InstNoOp(
                            name=f"{inst.name}-wsplit{k}", ins=[], outs=[]
                        )
                        nop.engine = inst.engine
                        nop.sync_info = mybir.SyncInfo(
                            on_wait=[w], on_update=[]
                        )
                        out.append(nop)
                    inst.sync_info = mybir.SyncInfo(
                        on_wait=[waits[-1]], on_update=list(si.on_update or [])
                    )
                    n_split += 1
                    changed = True
                out.append(inst)
            if changed:
                bb.instructions = out
    return n_split


def _dedup_ldweights(nc):
    """Tile legalization emits explicit Ldweights+Matmult pairs, and walrus
    runs with --enable-ldw-opt=false, so every matmul re-streams its
    stationary operand (128 extra PE cycles per matmul).  Drop an Ldweights
    whose weights AP is identical to the previous one on the PE stream (the
    PE array still holds that stationary); keep its semaphore effects on a
    NoOp."""
    n = 0
    for f in nc.m.functions:
        for bb in f.blocks:
            insts = bb.instructions
            out = []
            last_key = None
            changed = False
            for inst in insts:
                if isinstance(inst, mybir.InstLdweights):
                    key = str(inst.ins[0])
                    if key == last_key:
                        si = getattr(inst, "sync_info", None)
                        if si is not None and (si.on_wait or si.on_update):
                            nop = mybir.InstNoOp(
                                name=inst.name + "-lw", ins=[], outs=[]
                            )
                            nop.engine = inst.engine
                            nop.sync_info = si
                            out.append(nop)
                        n += 1
                        changed = True
                        continue
                    last_key = key
                elif isinstance(inst, mybir.InstMatmult):
                    if inst.is_transpose or getattr(inst, "ldweights", None):
                        last_key = None
                out.append(inst)
            if changed:
                bb.instructions = out
    return n


def build_nc(rows=4096, c=C, h=H, n_cores=N_CORES, gelu="Gelu",
             split_waits=True):
    """Build the per-core SPMD Bass program.

    rows: rows of x2d handled by each core.
    gelu: "Gelu" (HW ACT table), "Erf" (x*(0.5*erf(x/sqrt2)+0.5)),
          "Identity" (for simulator runs; CoreSim lacks Gelu/Erf).
    """
    assert sum(A_CHUNKS) == rows and c % 512 == 0 and h % 512 == 0
    nc = bass.Bass()

    x_in = nc.dram_tensor("x", [rows, c], F32, kind="ExternalInput")
    w1_in = nc.dram_tensor("w1", [h, c], F32, kind="ExternalInput")
    b1_in = nc.dram_tensor("b1", [h], F32, kind="ExternalInput")
    w2_in = nc.dram_tensor("w2", [c, h], F32, kind="ExternalInput")
    b2_in = nc.dram_tensor("b2", [c], F32, kind="ExternalInput")
    y_out = nc.dram_tensor("y", [rows, c], F32, kind="ExternalOutput")

    ct = c // 128    # 8
    ht = h // 128    # 32
    groups = [list(range(n_cores))]
    a_chunks = []
    m0 = 0
    for ln in A_CHUNKS:
        a_chunks.append((m0, ln))
        m0 += ln
    phb_chunks = []
    m0 = 0
    for ln in PHB_CHUNKS:
        phb_chunks.append((m0, ln))
        m0 += ln
    assert m0 == rows

    with tile.TileContext(nc) as tc, ExitStack() as top:
        consts = top.enter_context(tc.tile_pool(name="consts", bufs=1))
        scal = top.enter_context(tc.tile_pool(name="scal", bufs=1))
        dram = top.enter_context(tc.tile_pool(name="dram", bufs=1, space="DRAM"))
        rowp = top.enter_context(tc.tile_pool(name="rowp", bufs=2))

        ident = consts.tile([128, 128], FP16)
        masks.make_identity(nc, ident[:])
        # b1 as (128, ht): b1_sb[p, jb] = b1[jb*128 + p]
        b1_sb = consts.tile([128, ht], F32)
        nc.sync.dma_start(
            out=b1_sb[:], in_=b1_in.ap().rearrange("(a b) -> b a", b=128)
        )
        magic_b = consts.tile([128, 1], F32)
        nc.vector.memset(magic_b[:], MAGIC)
        junk_sb = consts.tile([128, 512], FP16)
        nc.vector.memset(junk_sb[:], 1.0)

        # DRAM scratch
        xq_dram = dram.tile([rows, c], FP16)
        g_dram = dram.tile([h, rows], FP16)
        w2q_dram = dram.tile([c, h], FP16)
        rt_dram = dram.tile([4, 128], F32, tag="rt")    # partition-red rows
        sc_dram = dram.tile([2, 1], F32, tag="sc")      # w1max, w2max scalars
        arx_in = dram.tile([1, 1], F32, tag="arxi")
        arx_out = dram.tile([1, 1], F32, tag="arxo")
        arh_in = dram.tile([1, 1], F32, tag="arhi")
        arh_out = dram.tile([1, 1], F32, tag="arho")

        def _preduce_dma(acc, slot, tag):
            """(128,1) partition max -> (1,1) via a tiny DRAM roundtrip on
            the gpsimd software DMA queue (PE and the HWDGE queues stay out
            of the scale critical path; walrus only lowers DMA on gpsimd)."""
            nc.gpsimd.dma_start(out=rt_dram[slot:slot + 1, :], in_=acc[:])
            row = rowp.tile([1, 128], F32, tag="rr", name=f"row_{tag}")
            nc.gpsimd.dma_start(out=row[:], in_=rt_dram[slot:slot + 1, :])
            s = scal.tile([1, 1], F32, name=f"s1_{tag}")
            nc.vector.tensor_reduce(
                out=s[:], in_=row[:], axis=mybir.AxisListType.X, op=ALU.max
            )
            return s

        def _derive(bcast_src_dram, name):
            """scale = max(amax,eps)/QP and its reciprocal (DVE; gpsimd only
            does the broadcast DMA)."""
            b = scal.tile([128, 1], F32, name=name + "_b")
            nc.gpsimd.dma_start(
                out=b[:], in_=bcast_src_dram.to_broadcast((128, 1))
            )
            s = scal.tile([128, 1], F32, name="s_" + name)
            nc.vector.tensor_scalar(
                out=s[:], in0=b[:], scalar1=EPS, scalar2=float(1.0 / QP),
                op0=ALU.max, op1=ALU.mult,
            )
            inv = scal.tile([128, 1], F32, name="inv_" + name)
            nc.vector.reciprocal(out=inv[:], in_=s[:])
            return s, inv

        def _junk(n, pspool, tag):
            """PE warmers: n chained 512-wide accumulating matmuls on a
            constant tile (ldweights dedup leaves one stationary load, so
            each costs ~515 PE cycles ~ 0.21us warm).  Keeps the HAM clock
            gate at K=8 while real work is DMA/collective-bound.  The psum
            is never read -- it only exists to occupy the PE."""
            ps = pspool.tile([128, 512], F32, tag=tag, name=f"ps_{tag}")
            prev = None
            for i in range(n):
                mmi = nc.tensor.matmul(
                    ps[:], lhsT=junk_sb[:, 0:128], rhs=junk_sb[:],
                    start=(i == 0), stop=(i == n - 1),
                    skip_group_check=True,
                )
                if prev is not None:
                    _add_dep(mmi.ins, prev.ins, sync=False, reason="junk-order")
                prev = mmi
            return prev

        def _amax_tile(tl, acc, tag):
            """DVE bulk |max| reduce + gpsimd accumulation into acc."""
            r = rowp.tile([128, 1], F32, tag="red", name=f"r_{tag}")
            nc.vector.tensor_reduce(
                out=r[:], in_=tl[:], axis=mybir.AxisListType.X, op=ALU.max,
                apply_absolute_value=True,
            )
            nc.vector.tensor_tensor(out=acc[:], in0=acc[:], in1=r[:],
                                    op=ALU.max)

        # long-lived (front .. end of phase A)
        a_stack = ExitStack()
        w1qT_pool = a_stack.enter_context(
            tc.tile_pool(name="w1qT", bufs=1, side="right")
        )
        w1qT = [
            w1qT_pool.tile([128, h], FP16, tag=f"w1qT{i}", name=f"w1qT{i}")
            for i in range(ct)
        ]
        # xqT small chunks (1024 rows) double-buffered; big chunk separate
        xqTs_pool = a_stack.enter_context(tc.tile_pool(name="xqTs", bufs=2))
        xf = a_stack.enter_context(tc.tile_pool(name="xf", bufs=8))
        xq1 = a_stack.enter_context(tc.tile_pool(name="xq1", bufs=2))

        def emit_xq(mc, t, ld_eng, st_eng):
            """Quantize x row-block t of chunk mc: f32 load was already
            issued into xf; ACT magic-round in place; DVE -> fp16; stage to
            xq_dram on st_eng's queue."""
            cm0, clen = a_chunks[mc]
            r0 = cm0 + t * 128
            xt = xf.tile([128, c], F32, tag="xf", name=f"xf{mc}_{t}")
            ld_eng.dma_start(out=xt[:], in_=x_in[r0:r0 + 128, :])
            nc.scalar.activation(
                out=xt[:], in_=xt[:], func=AF.Identity,
                bias=magic_b[:], scale=inv_sx[:],
            )
            q = xq1.tile([128, c], FP16, tag="xq1", name=f"xq{mc}_{t}")
            nc.vector.tensor_scalar_add(out=q[:], in0=xt[:], scalar1=-MAGIC)
            st_eng.dma_start(out=xq_dram[r0:r0 + 128, :], in_=q[:])

        def emit_dmat(mc, qt, xqTs, eng):
            """XBAR-transpose 512-row quarter qt of chunk mc into xqTs."""
            cm0, clen = a_chunks[mc]
            r0 = cm0 + qt * 512
            for cb in range(ct):
                eng.dma_start_transpose(
                    xqTs[cb][:, qt * 512:(qt + 1) * 512],
                    xq_dram[r0:r0 + 512, cb * 128:(cb + 1) * 128],
                )

        def new_xqTs(mc, pool):
            cm0, clen = a_chunks[mc]
            return [
                pool.tile([128, clen], FP16, tag=f"xqT{cb}",
                          name=f"xqT{mc}_{cb}")
                for cb in range(ct)
            ]

        # ---------------- front ----------------
        xmax = scal.tile([128, 1], F32)
        nc.vector.memset(xmax[:], 0.0)
        wmax1 = scal.tile([128, 1], F32)
        nc.vector.memset(wmax1[:], 0.0)

        with ExitStack() as front:
            psJ = front.enter_context(
                tc.tile_pool(name="psJ", bufs=1, space="PSUM")
            )
            xs = front.enter_context(tc.tile_pool(name="xs", bufs=2))
            w1f_pool = front.enter_context(tc.tile_pool(name="w1f", bufs=1))
            wq1 = front.enter_context(tc.tile_pool(name="wq1", bufs=2))
            psT = front.enter_context(
                tc.tile_pool(name="psT", bufs=4, space="PSUM")
            )

            _junk(JUNK_FRONT, psJ, "junkF")

            # x amax scan FIRST, striped over both HWDGE queues; its
            # AllReduce hides under the w1 load that follows.
            for t in range(rows // 128):
                eng = nc.sync if t % 2 == 0 else nc.scalar
                xt = xs.tile([128, c], F32, tag="xs", name=f"xs{t}")
                eng.dma_start(out=xt[:], in_=x_in[t * 128:(t + 1) * 128, :])
                _amax_tile(xt, xmax, f"x{t}")
            xm_s = _preduce_dma(xmax, 0, "xm")
            nc.gpsimd.dma_start(out=arx_in[:], in_=xm_s[:])
            nc.gpsimd.collective_compute(
                "AllReduce", ALU.max, replica_groups=groups,
                ins=[arx_in.opt()], outs=[arx_out.opt()],
            )

            # w1 load (f32-resident where SBUF allows) + amax; every core
            # scans the FULL weights so the local max is already global.
            w1res = []
            for t in range(ht):
                eng = nc.sync if t % 2 == 0 else nc.scalar
                if t < W1_RES:
                    wt = w1f_pool.tile([128, c], F32, tag=f"w1f{t}",
                                       name=f"w1f{t}")
                    w1res.append(wt)
                else:
                    wt = xs.tile([128, c], F32, tag="xs", name=f"w1s{t}")
                eng.dma_start(out=wt[:], in_=w1_in[t * 128:(t + 1) * 128, :])
                _amax_tile(wt, wmax1, f"w1{t}")
            w1m_s = _preduce_dma(wmax1, 1, "w1m")
            nc.gpsimd.dma_start(out=sc_dram[0:1, :], in_=w1m_s[:])
            sw1, inv_sw1 = _derive(sc_dram[0:1, :], "w1")
            sx, inv_sx = _derive(arx_out, "x")
            sxw1 = scal.tile([128, 1], F32)
            nc.vector.tensor_tensor(out=sxw1[:], in0=sx[:], in1=sw1[:],
                                    op=ALU.mult)

            # interleaved: w1 quant+PE-transpose with chunk-0 x quant, so
            # ACT alternates between them and the first matmuls unblock at
            # max(sx, sw1) + a few tiles.
            xqTs0 = new_xqTs(0, xqTs_pool)
            n_xq0 = a_chunks[0][1] // 128
            for t in range(ht):
                if t < W1_RES:
                    src = w1res[t]
                else:
                    src = xs.tile([128, c], F32, tag="xs", name=f"w1q_s{t}")
                    nc.sync.dma_start(
                        out=src[:], in_=w1_in[t * 128:(t + 1) * 128, :]
                    )
                nc.scalar.activation(
                    out=src[:], in_=src[:], func=AF.Identity, bias=magic_b[:],
                    scale=inv_sw1[:],
                )
                q1 = wq1.tile([128, c], FP16, tag="wq1", name=f"w1q1_{t}")
                nc.vector.tensor_scalar_add(out=q1[:], in0=src[:],
                                            scalar1=-MAGIC)
                for cb in range(ct):
                    ps = psT.tile([128, 128], F32, tag="psT",
                                  name=f"psT{t}_{cb}")
                    nc.tensor.matmul(
                        ps[:], lhsT=q1[:, cb * 128:(cb + 1) * 128],
                        rhs=ident[:], start=True, stop=True,
                    )
                    nc.vector.tensor_copy(
                        out=w1qT[cb][:, t * 128:(t + 1) * 128], in_=ps[:]
                    )
                if t < n_xq0:
                    emit_xq(0, t, nc.scalar, nc.scalar)
                    if t % 4 == 3:
                        emit_dmat(0, t // 4, xqTs0, nc.scalar)

        # ---------------- phase A ----------------
        hmax = scal.tile([128, 1], F32)
        nc.vector.memset(hmax[:], 0.0)
        wmax2 = scal.tile([128, 1], F32)
        nc.vector.memset(wmax2[:], 0.0)
        sw2 = inv_sw2 = None
        n_w2t = 16  # w2 row tiles (128, 2048)

        with ExitStack() as pha:
            xqTb_pool = pha.enter_context(tc.tile_pool(name="xqTb", bufs=1))
            psH = pha.enter_context(
                tc.tile_pool(name="psH", bufs=8, space="PSUM")
            )
            gS = pha.enter_context(tc.tile_pool(name="gS", bufs=3))
            w2f = pha.enter_context(tc.tile_pool(name="w2f", bufs=2))
            w2qs = pha.enter_context(tc.tile_pool(name="w2qs", bufs=2))

            def emit_w2_scan(t):
                blk, hf = t // 2, t % 2
                wt = w2f.tile([128, 2048], F32, tag="w2f", name=f"w2s{t}")
                nc.scalar.dma_start(
                    out=wt[:],
                    in_=w2_in[blk * 128:(blk + 1) * 128,
                              hf * 2048:(hf + 1) * 2048],
                )
                _amax_tile(wt, wmax2, f"w2{t}")

            def emit_w2_quant(t):
                blk, hf = t // 2, t % 2
                wt = w2f.tile([128, 2048], F32, tag="w2f", name=f"w2qs{t}")
                nc.scalar.dma_start(
                    out=wt[:],
                    in_=w2_in[blk * 128:(blk + 1) * 128,
                              hf * 2048:(hf + 1) * 2048],
                )
                nc.scalar.activation(
                    out=wt[:], in_=wt[:], func=AF.Identity,
                    bias=magic_b[:], scale=inv_sw2[:],
                )
                q = w2qs.tile([128, 2048], FP16, tag="w2qs", name=f"w2q{t}")
                nc.vector.tensor_scalar_add(out=q[:], in0=wt[:],
                                            scalar1=-MAGIC)
                nc.scalar.dma_start(
                    out=w2q_dram[blk * 128:(blk + 1) * 128,
                                 hf * 2048:(hf + 1) * 2048],
                    in_=q[:],
                )

            last = len(a_chunks) - 1
            for mc, (cm0, clen) in enumerate(a_chunks):
                n_ms = clen // 512
                if mc == 0:
                    xqTs = xqTs0  # built during the front
                else:
                    pool = xqTb_pool if mc == last else xqTs_pool
                    xqTs = new_xqTs(mc, pool)
                    for t in range(clen // 128):
                        emit_xq(mc, t, nc.sync, nc.sync)
                        if t % 4 == 3:
                            emit_dmat(mc, t // 4, xqTs, nc.sync)
                # w2 trickle: scan over chunks 0-1, quant inside the last
                # chunk's jb loop (interleaved so ACT never bursts)
                if mc < last:
                    for t in range(n_w2t // 2):
                        emit_w2_scan(mc * (n_w2t // 2) + t)
                else:
                    w2m_s = _preduce_dma(wmax2, 2, "w2m")
                    nc.gpsimd.dma_start(out=sc_dram[1:2, :], in_=w2m_s[:])
                    sw2, inv_sw2 = _derive(sc_dram[1:2, :], "w2")

                for jb in range(ht):
                    phs = [
                        psH.tile([128, 512], F32, tag="psH",
                                 name=f"psH{mc}_{jb}_{i}")
                        for i in range(n_ms)
                    ]
                    prev = None
                    for cb in range(ct):
                        for ms in range(n_ms):
                            mmi = nc.tensor.matmul(
                                phs[ms][:],
                                lhsT=w1qT[cb][:, jb * 128:(jb + 1) * 128],
                                rhs=xqTs[cb][:, ms * 512:(ms + 1) * 512],
                                start=(cb == 0),
                                stop=(cb == ct - 1),
                            )
                            if prev is not None:
                                _add_dep(mmi.ins, prev.ins, sync=False,
                                         reason="ldw-order")
                            prev = mmi
                    g = gS.tile([128, clen], FP16, tag="gS",
                                name=f"g{mc}_{jb}")
                    for ms in range(n_ms):
                        if gelu == "Erf":
                            hh = gS.tile([128, 512], F32, tag="gHH",
                                         name=f"hh{mc}_{jb}_{ms}")
                            nc.scalar.activation(
                                out=hh[:], in_=phs[ms][:], func=AF.Identity,
                                bias=b1_sb[:, jb:jb + 1], scale=sxw1[:],
                            )
                            e = gS.tile([128, 512], F32, tag="gE",
                                        name=f"e{mc}_{jb}_{ms}")
                            nc.scalar.activation(
                                out=e[:], in_=hh[:], func=AF.Erf, bias=0.0,
                                scale=float(1.0 / np.sqrt(2.0)),
                            )
                            nc.vector.tensor_scalar(
                                out=e[:], in0=e[:], scalar1=0.5, scalar2=0.5,
                                op0=ALU.mult, op1=ALU.add,
                            )
                            nc.vector.tensor_tensor(
                                out=g[:, ms * 512:(ms + 1) * 512], in0=e[:],
                                in1=hh[:], op=ALU.mult,
                            )
                        else:
                            nc.scalar.activation(
                                out=g[:, ms * 512:(ms + 1) * 512],
                                in_=phs[ms][:], func=getattr(AF, gelu),
                                bias=b1_sb[:, jb:jb + 1], scale=sxw1[:],
                            )
                    _amax_tile(g, hmax, f"g{mc}_{jb}")
                    nc.sync.dma_start(
                        out=g_dram[jb * 128:(jb + 1) * 128, cm0:cm0 + clen],
                        in_=g[:],
                    )
                    if mc == last and jb % 2 == 0 and jb // 2 < n_w2t:
                        emit_w2_quant(jb // 2)

        a_stack.close()

        # ---------------- h scale AllReduce + transition ----------------
        hm_s = _preduce_dma(hmax, 3, "hm")
        nc.gpsimd.dma_start(out=arh_in[:], in_=hm_s[:])
        nc.gpsimd.collective_compute(
            "AllReduce", ALU.max, replica_groups=groups,
            ins=[arh_in.opt()], outs=[arh_out.opt()],
        )

        # ---------------- phase B ----------------
        with ExitStack() as phb:
            psY = phb.enter_context(
                tc.tile_pool(name="psY", bufs=6, space="PSUM")
            )
            psJ2 = phb.enter_context(
                tc.tile_pool(name="psJ2", bufs=1, space="PSUM")
            )
            w2qT_pool = phb.enter_context(tc.tile_pool(name="w2qT", bufs=1))
            hld = phb.enter_context(tc.tile_pool(name="hld", bufs=2))
            hq1p = phb.enter_context(tc.tile_pool(name="hq1p", bufs=3))
            yS = phb.enter_context(tc.tile_pool(name="yS", bufs=3))
            b2p = phb.enter_context(tc.tile_pool(name="b2p", bufs=1))

            # PE warmers across the AllReduce bubble (emitted after the last
            # phase-A matmul in PE program order)
            _junk(JUNK_MID, psJ2, "junkM")

            b2_b = b2p.tile([128, c], F32)
            nc.sync.dma_start(
                out=b2_b[:],
                in_=b2_in.ap().rearrange("(o a) -> o a", o=1).to_broadcast(
                    (128, c)),
            )
            # w2qT XBAR transposes on the SCALAR queue so the phase-B h
            # loads (sync queue) are not stuck behind them
            w2qT = [
                w2qT_pool.tile([128, c], FP16, tag=f"w2qT{jb}",
                               name=f"w2qT{jb}")
                for jb in range(ht)
            ]
            for jb in range(ht):
                nc.scalar.dma_start_transpose(
                    w2qT[jb][:], w2q_dram[:, jb * 128:(jb + 1) * 128]
                )

            sh, inv_sh = _derive(arh_out, "h")
            shw2 = scal.tile([128, 1], F32)
            nc.vector.tensor_tensor(out=shw2[:], in0=sh[:], in1=sw2[:],
                                    op=ALU.mult)

            for ci, (m0, mlen) in enumerate(phb_chunks):
                hl = hld.tile([128, ht, 512], FP16, tag="hld",
                              name=f"hl{ci}")
                hlv = hl[:, :, 0:mlen]
                nc.sync.dma_start(
                    out=hlv,
                    in_=g_dram[:, m0:m0 + mlen].rearrange(
                        "(a p) m -> p a m", p=128),
                )
                for j4 in range(ht // 4):
                    sl = hl[:, j4 * 4:(j4 + 1) * 4, 0:mlen]
                    hq1 = hq1p.tile([128, 4, 512], F32, tag="hq1",
                                    name=f"hq1_{ci}_{j4}")
                    nc.scalar.activation(
                        out=hq1[:, :, 0:mlen], in_=sl, func=AF.Identity,
                        bias=magic_b[:], scale=inv_sh[:],
                    )
                    nc.vector.tensor_scalar_add(
                        out=sl, in0=hq1[:, :, 0:mlen], scalar1=-MAGIC
                    )
                for ms in range(mlen // 128):
                    psa = psY.tile([128, 512], F32, tag="psY",
                                   name=f"psa{ci}_{ms}")
                    psb = psY.tile([128, 512], F32, tag="psY",
                                   name=f"psb{ci}_{ms}")
                    prev = None
                    for jb in range(ht):
                        lt = hl[:, jb:jb + 1, ms * 128:(ms + 1) * 128]
                        for ob, pso in ((0, psa), (1, psb)):
                            mmi = nc.tensor.matmul(
                                pso[:], lhsT=lt,
                                rhs=w2qT[jb][:, ob * 512:(ob + 1) * 512],
                                start=(jb == 0), stop=(jb == ht - 1),
                            )
                            if prev is not None:
                                _add_dep(mmi.ins, prev.ins, sync=False,
                                         reason="ldw-order")
                            prev = mmi
                    yt = yS.tile([128, c], F32, tag="yS", name=f"y{ci}_{ms}")
                    nc.vector.scalar_tensor_tensor(
                        out=yt[:, 0:512], in0=psa[:], scalar=shw2[:],
                        in1=b2_b[:, 0:512], op0=ALU.mult, op1=ALU.add,
                    )
                    nc.vector.scalar_tensor_tensor(
                        out=yt[:, 512:1024], in0=psb[:], scalar=shw2[:],
                        in1=b2_b[:, 512:1024], op0=ALU.mult, op1=ALU.add,
                    )
                    r0 = m0 + ms * 128
                    nc.sync.dma_start(out=y_out[r0:r0 + 128, :], in_=yt[:])

    if split_waits:
        _split_matmul_waits(nc)
        _dedup_ldweights(nc)
    return nc


_CACHED = {}


def _get_nc(rows, c, h, n_cores, gelu):
    key = (rows, c, h, n_cores, gelu)
    if key not in _CACHED:
        _CACHED[key] = build_nc(rows=rows, c=c, h=h, n_cores=n_cores,
                                gelu=gelu)
    return _CACHED[key]


def run(inputs, trace=False, gelu="Gelu", n_cores=N_CORES):
    x = np.asarray(inputs["x"], np.float32)
    w1 = np.ascontiguousarray(np.asarray(inputs["w1"], np.float32))
    b1 = np.ascontiguousarray(np.asarray(inputs["b1"], np.float32))
    w2 = np.ascontiguousarray(np.asarray(inputs["w2"], np.float32))
    b2 = np.ascontiguousarray(np.asarray(inputs["b2"], np.float32))
    b_, s_, c_ = x.shape
    h_ = w1.shape[0]
    x2d = np.ascontiguousarray(x.reshape(-1, c_))
    rows = x2d.shape[0] // n_cores
    nc = _get_nc(rows, c_, h_, n_cores, gelu)
    in_maps = [
        {
            "x": np.ascontiguousarray(x2d[i * rows:(i + 1) * rows]),
            "w1": w1,
            "b1": b1,
            "w2": w2,
            "b2": b2,
        }
        for i in range(n_cores)
    ]
    res = run_bass_kernel_spmd(nc, in_maps, list(range(n_cores)), trace=trace)
    y2d = np.concatenate([r["y"] for r in res.results], axis=0)
    return y2d.reshape(b_, s_, c_).astype(np.float32), res


def kernel(x, w1, b1, w2, b2):
    y, _ = run({"x": x, "w1": w1, "b1": b1, "w2": w2, "b2": b2})
    return y
